# revision 2
# baseline (speedup 1.0000x reference)
"""CNOT permutation kernel for Trainium2 (Bass), 8-core data parallel.

Problem (hardcoded from spec): state (16, 2**24) f32, control=3, target=10,
num_qubits=24.  With c2 = 24-3-1 = 20 and t2 = 24-10-1 = 13:

    out[b, j] = state[b, j ^ (1<<13)]  if (j >> 20) & 1 else state[b, j]

This is pure data movement.  Viewing a row as [a:8, ctrl:2, c:64, d:2, e:8192]
(index bits [23:21][20][19:14][13][12:0]):

    out[a, 0, c, d, e] = in[a, 0, c, d, e]        (identity half)
    out[a, 1, c, d, e] = in[a, 1, c, 1-d, e]      (swap 8192-elem chunk pairs)

Sharding: batch axis, 2 rows per core (pure data parallel; the permutation
only mixes amplitudes within a row).  Because the row stride (2**24) is a
multiple of the block stride (2**21), both rows of a core's shard fuse into
single access patterns: the whole per-core job is 3 DRAM->DRAM DMAs:

    D1 identity: 64 MiB, AP [[2**21, 16], [1, 2**20]]
    D2 d0<-d1:   32 MiB, AP [[2**21, 16], [2**14, 64], [1, 2**13]] (+-8192 offs)
    D3 d1<-d0:   32 MiB, mirrored

No SBUF, no compute.  HBM-bound: ~256 MiB traffic/core @ ~358 GB/s.
"""

import numpy as np

import concourse.bass as bass
import concourse.mybir as mybir
from concourse.bass_utils import run_bass_kernel_spmd
from concourse.tile import TileContext

NUM_QUBITS = 24
DIM = 1 << NUM_QUBITS
BATCH = 16
N_CORES = 8
ROWS = BATCH // N_CORES  # 2 rows per core
C2 = NUM_QUBITS - 3 - 1  # 20
T2 = NUM_QUBITS - 10 - 1  # 13
CBIT = 1 << C2  # 1048576
TBIT = 1 << T2  # 8192
BLK = CBIT * 2  # 2097152, period of the control-bit pattern
NBLK = ROWS * DIM // BLK  # 16 blocks across the fused (rows, dim) space

_nc_cache = None


def _build_nc():
    nc = bass.Bass(target_bir_lowering=False)
    st = nc.dram_tensor("state", (ROWS, DIM), mybir.dt.float32, kind="ExternalInput")
    out = nc.dram_tensor("out", (ROWS, DIM), mybir.dt.float32, kind="ExternalOutput")

    swap_ap = [[BLK, NBLK], [2 * TBIT, CBIT // (2 * TBIT)], [1, TBIT]]
    with nc.semaphore("dsem") as dsem, nc.Block() as block:

        @block.sync
        def _(sync):
            # identity half (control bit 0)
            sync.dma_start(
                out=bass.AP(out, 0, [[BLK, NBLK], [1, CBIT]]),
                in_=bass.AP(st, 0, [[BLK, NBLK], [1, CBIT]]),
            ).then_inc(dsem, 16)
            # swap half (control bit 1): out d=0 <- in d=1
            sync.dma_start(
                out=bass.AP(out, CBIT, [r[:] for r in swap_ap]),
                in_=bass.AP(st, CBIT + TBIT, [r[:] for r in swap_ap]),
            ).then_inc(dsem, 16)
            # swap half: out d=1 <- in d=0
            sync.dma_start(
                out=bass.AP(out, CBIT + TBIT, [r[:] for r in swap_ap]),
                in_=bass.AP(st, CBIT, [r[:] for r in swap_ap]),
            ).then_inc(dsem, 16)
            sync.wait_ge(dsem, 48)

    if not nc.is_finalized():
        nc.finalize()
    return nc


def kernel(state, control=3, target=10, num_qubits=24, **_):
    global _nc_cache
    state = np.ascontiguousarray(np.asarray(state, dtype=np.float32))
    assert state.shape == (BATCH, DIM), state.shape
    assert int(control) == 3 and int(target) == 10 and int(num_qubits) == 24

    if _nc_cache is None:
        _nc_cache = _build_nc()
    nc = _nc_cache

    in_maps = [
        {"state": state[c * ROWS : (c + 1) * ROWS]} for c in range(N_CORES)
    ]
    res = run_bass_kernel_spmd(nc, in_maps, core_ids=list(range(N_CORES)))
    return np.concatenate([r["out"] for r in res.results], axis=0)


# revision 3
# speedup vs baseline: 1.8757x; 1.8757x over previous
"""CNOT permutation kernel for Trainium2 (Bass), 8-core data parallel.

Problem (hardcoded from spec): state (16, 2**24) f32, control=3, target=10,
num_qubits=24.  With c2 = 24-3-1 = 20 and t2 = 24-10-1 = 13:

    out[b, j] = state[b, j ^ (1<<13)]  if (j >> 20) & 1 else state[b, j]

Pure data movement.  Viewing the per-core shard flat (row stride 2**24 is a
multiple of the 2**21 control-bit period, so both rows fuse) as
[blk:16, ctrl:2, c:64, d:2, e:8192]:

    out[blk, 0, c, d, e] = in[blk, 0, c, d, e]      (identity half)
    out[blk, 1, c, d, e] = in[blk, 1, c, 1-d, e]    (swap 8192-elem chunk pairs)

Sharding: batch axis, 2 rows per core (pure data parallel).

Implementation: HBM->SBUF->HBM bounce.  Direct DRAM->DRAM DMA measures only
~5.8 GB/s per SDMA engine (read/write serialize inside the engine), while
HBM->SBUF and SBUF->HBM stream near line rate.  Loads issue on the Sync
HWDGE ring, stores on the Scalar HWDGE ring; the 16 SDMA engines round-robin
between the two rings at packet granularity, so both HBM directions stay
busy.  4 MiB slabs ([128, 8192] f32 tiles), NBUF-deep manual double
buffering with two semaphores (RAW: store waits load; WAR: load waits the
store that previously used its slot).  The chunk-pair swap is done in the
store APs: even partitions (d=0 data) store to d=1 positions and vice versa.
"""

import numpy as np

import concourse.bass as bass
import concourse.mybir as mybir
from concourse.bass_utils import run_bass_kernel_spmd

NUM_QUBITS = 24
DIM = 1 << NUM_QUBITS
BATCH = 16
N_CORES = 8
ROWS = BATCH // N_CORES  # 2 rows per core
C2 = NUM_QUBITS - 3 - 1  # 20
T2 = NUM_QUBITS - 10 - 1  # 13
CBIT = 1 << C2  # 1048576 elements (4 MiB)
TBIT = 1 << T2  # 8192 elements (32 KiB)
BLK = 2 * CBIT  # control-bit period
NBLK = ROWS * DIM // BLK  # 16 blocks in the fused per-core space

P = 128
FREE = CBIT // P  # 8192: slab is [128, 8192] f32 = 4 MiB
NBUF = 6

_nc_cache = None


def _build_nc():
    nc = bass.Bass(target_bir_lowering=False)
    st = nc.dram_tensor("state", (ROWS, DIM), mybir.dt.float32, kind="ExternalInput")
    out = nc.dram_tensor("out", (ROWS, DIM), mybir.dt.float32, kind="ExternalOutput")

    # slabs: (base_offset, is_swap) — identity half then swap half of each block
    slabs = []
    for b in range(NBLK):
        slabs.append((b * BLK, False))
        slabs.append((b * BLK + CBIT, True))
    n = len(slabs)  # 32

    with (
        nc.sbuf_tensor("tiles", [P, NBUF * FREE], mybir.dt.float32) as tiles,
        nc.semaphore("load_sem") as load_sem,
        nc.semaphore("store_sem") as store_sem,
        nc.Block() as block,
    ):

        def tile_view(i):
            return tiles[:, (i % NBUF) * FREE : (i % NBUF + 1) * FREE]

        @block.sync
        def _(sync):
            for i, (base, _swap) in enumerate(slabs):
                if i >= NBUF:
                    sync.wait_ge(store_sem, 32 * (i - NBUF + 1))
                sync.dma_start(
                    out=tile_view(i),
                    in_=bass.AP(st, base, [[1, CBIT]]),
                ).then_inc(load_sem, 16)

        @block.scalar
        def _(scalar):
            for i, (base, swap) in enumerate(slabs):
                scalar.wait_ge(load_sem, 16 * (i + 1))
                t = tile_view(i)
                if swap:
                    # partition p holds chunk (c, d) with p = 2c + d
                    even = t[0::2, :]  # d=0 data -> d=1 positions
                    odd = t[1::2, :]  # d=1 data -> d=0 positions
                    scalar.dma_start(
                        out=bass.AP(out, base + TBIT, [[2 * TBIT, P // 2], [1, TBIT]]),
                        in_=even,
                    ).then_inc(store_sem, 16)
                    scalar.dma_start(
                        out=bass.AP(out, base, [[2 * TBIT, P // 2], [1, TBIT]]),
                        in_=odd,
                    ).then_inc(store_sem, 16)
                else:
                    scalar.dma_start(
                        out=bass.AP(out, base, [[1, CBIT // 2]]),
                        in_=t[0 : P // 2, :],
                    ).then_inc(store_sem, 16)
                    scalar.dma_start(
                        out=bass.AP(out, base + CBIT // 2, [[1, CBIT // 2]]),
                        in_=t[P // 2 :, :],
                    ).then_inc(store_sem, 16)
            scalar.wait_ge(store_sem, 32 * n)

    if not nc.is_finalized():
        nc.finalize()
    return nc


def kernel(state, control=3, target=10, num_qubits=24, **_):
    global _nc_cache
    state = np.ascontiguousarray(np.asarray(state, dtype=np.float32))
    assert state.shape == (BATCH, DIM), state.shape
    assert int(control) == 3 and int(target) == 10 and int(num_qubits) == 24

    if _nc_cache is None:
        _nc_cache = _build_nc()
    nc = _nc_cache

    in_maps = [
        {"state": state[c * ROWS : (c + 1) * ROWS]} for c in range(N_CORES)
    ]
    res = run_bass_kernel_spmd(nc, in_maps, core_ids=list(range(N_CORES)))
    return np.concatenate([r["out"] for r in res.results], axis=0)
